# revision 25
# baseline (speedup 1.0000x reference)
"""GAT layer (project + edge-softmax attention + aggregate + head-mean + LayerNorm + PReLU)
on 8 Trainium2 NeuronCores.

Sharding: nodes/edges partitioned by destination across the 8 cores; edges of
each core are grouped into 128-destination blocks and 128-edge tiles.

Device pipeline (v4):
 - host ships, per edge slot: source features x (fp16, the 99.9%-of-FLOPs
   projection runs on device), the fp8 one-hot destination mask (exact 0/1),
   and the folded attention logit a_src+a_dst (fp16, 8B) pre-gathered the same
   way the features are.
 - per 48-tile chunk: one DVE leaky-relu + one ACT exp produce the edge exp
   weights.
 - per 6-tile PSUM group: 6 projection matmuls (xet.T @ W, fp16); the h*e
   multiply splits DVE (4 tiles, fused PSUM->SBUF) / ACT copy + GpSimd
   multiply (2 tiles); softmax denominators ride as 4 extra rhs columns
   (strided GpSimd copy of e).
 - per tile one 260-wide aggregation matmul (fp8 mask stationary x fp16
   moving) accumulating per dst block in a double-buffered PSUM bank; ACT
   copies finished blocks out.
 - the tail epilogue (head-mean with per-(dst,head) softmax denominators,
   LayerNorm, PReLU) splits across DVE/GpSimd by block range, with ACT doing
   the broadcasts/exp-like ops; output is written contiguously and
   de-interleaved on the host.
"""
import sys

sys.path.insert(0, "/opt/trn_rl_repo")

import numpy as np
from contextlib import ExitStack

import concourse.bass as bass
import concourse.tile as tile
from concourse import bacc, mybir
from concourse.bass_utils import run_bass_kernel_spmd

# ---- problem constants (hardcoded per harness contract) ----
N = 50000
IN_DIM = 128
OUT_DIM = 64
HEADS = 4
HC = HEADS * OUT_DIM          # 256
NEG_SLOPE = 0.2
EPS = 1e-5

NCORES = 8
ND = N // NCORES              # 6250 dst nodes per core
P = 128
NB = (ND + P - 1) // P        # 49 blocks (last has 106 dsts)
NDP = NB * P                  # 6272 padded local nodes
G = 6                         # tiles per PSUM projection group (3 banks)
CH = 48                       # tiles per alpha chunk (multiple of G)

F8 = mybir.dt.float8e4
F16 = mybir.dt.float16
F32 = mybir.dt.float32
NP_F8 = mybir.dt.np(F8)

_CACHE = {}


def _build(S, T_b):
    """Compile the SPMD program. S = padded edge slots per core (mult of 128),
    T_b = tuple of per-block tile counts (len NB, sum*128 == S)."""
    n_tiles = S // P
    RW = HC + HEADS           # 260 psum width (256 msg + 4 denom cols)

    nc = bacc.Bacc("TRN2", target_bir_lowering=False, debug=False)

    xeT = nc.dram_tensor("xeT", [P, S], F16, kind="ExternalInput")
    smaskd = nc.dram_tensor("smask", [P, S], F8, kind="ExternalInput")
    alphad = nc.dram_tensor("alphaT", [P, (S // P) * HEADS], F16,
                            kind="ExternalInput")
    W16d = nc.dram_tensor("W16", [P, HC], F16, kind="ExternalInput")
    # packed per-channel constants replicated across partitions:
    # [bias(64) | gamma(64) | beta(64) | prelu_w(1)]
    crep = nc.dram_tensor("crep", [P, 3 * OUT_DIM + 1], F32, kind="ExternalInput")
    # contiguous output dump [p, b, c]; host de-interleaves
    out = nc.dram_tensor("out", [P, NB * OUT_DIM], F16, kind="ExternalOutput")

    # tile -> (block, is_first_in_block, is_last_in_block)
    tinfo = []
    for b, nt in enumerate(T_b):
        for ti in range(nt):
            tinfo.append((b, ti == 0, ti == nt - 1))

    with tile.TileContext(nc) as tc, ExitStack() as ctx:
        const_p = ctx.enter_context(tc.tile_pool(name="const", bufs=1))
        xet_p = ctx.enter_context(tc.tile_pool(name="xet", bufs=2))
        rhs_p = ctx.enter_context(tc.tile_pool(name="rhs", bufs=4))
        ach_p = ctx.enter_context(tc.tile_pool(name="ach", bufs=2))
        epi_p = ctx.enter_context(tc.tile_pool(name="epi", bufs=1))
        ph_p = ctx.enter_context(tc.tile_pool(name="ph", bufs=2, space="PSUM"))
        pm_p = ctx.enter_context(tc.tile_pool(name="pm", bufs=2, space="PSUM"))

        # ---- constants ----
        w_s = const_p.tile([P, HC], F16)
        nc.sync.dma_start(w_s[:], W16d[:])
        cr_s = const_p.tile([P, 3 * OUT_DIM + 1], F32)
        nc.sync.dma_start(cr_s[:], crep[:])
        w_prelu = cr_s[:, 3 * OUT_DIM:3 * OUT_DIM + 1]

        # big accumulators for the batched epilogue (head-major for
        # contiguous tail ops)
        acc_h = const_p.tile([P, HEADS * NB * OUT_DIM], F16)
        acc_den = const_p.tile([P, NB * HEADS], F16)

        # ---- main loop (software-pipelined) ----
        nchunks = (n_tiles + CH - 1) // CH

        # groups spanning all chunks: (chunk, tile offset in chunk, size)
        groups = []
        for c in range(nchunks):
            ctiles = min(CH, n_tiles - c * CH)
            for g0 in range(0, ctiles, G):
                groups.append((c, g0, min(G, ctiles - g0)))

        chunk_st = {}

        def emit_dma(c):
            ctiles = min(CH, n_tiles - c * CH)
            lo, hi = c * CH * P, (c * CH + ctiles) * P
            w = hi - lo
            xet_ch = xet_p.tile([P, CH * P], F16, tag="xet")
            nc.sync.dma_start(xet_ch[:, :w], xeT[:, lo:hi])
            sm_ch = xet_p.tile([P, CH * P], F8, tag="smask")
            nc.sync.dma_start(sm_ch[:, :w], smaskd[:, lo:hi])
            al_ch = xet_p.tile([P, CH * HEADS], F16, tag="alpha")
            nc.sync.dma_start(al_ch[:, :ctiles * HEADS],
                              alphad[:, c * CH * HEADS:
                                     (c * CH + ctiles) * HEADS])
            chunk_st[c] = [xet_ch, sm_ch, al_ch, None]

        def emit_exp(c):
            ctiles = min(CH, n_tiles - c * CH)
            al_ch = chunk_st[c][2]
            cw = ctiles * HEADS
            lk_ch = ach_p.tile([P, CH * HEADS], F32, tag="lk_ch")
            nc.vector.scalar_tensor_tensor(
                out=lk_ch[:, :cw], in0=al_ch[:, :cw],
                scalar=NEG_SLOPE, in1=al_ch[:, :cw],
                op0=mybir.AluOpType.mult, op1=mybir.AluOpType.max)
            e_ch = ach_p.tile([P, CH * HEADS], F16, tag="e_ch")
            nc.scalar.activation(e_ch[:, :cw], lk_ch[:, :cw],
                                 mybir.ActivationFunctionType.Exp)
            chunk_st[c][3] = e_ch

        ph_of = {}
        rhs_of = {}

        def emit_ph(gi):
            c, g0, gsz = groups[gi]
            xet_ch = chunk_st[c][0]
            ph6 = ph_p.tile([P, G * HC], F32, space="PSUM", tag="ph6")
            for ti in range(gsz):
                nc.tensor.matmul(
                    ph6[:, ti * HC:(ti + 1) * HC],
                    lhsT=xet_ch[:, (g0 + ti) * P:(g0 + ti + 1) * P],
                    rhs=w_s[:], start=True, stop=True,
                    skip_group_check=True)
            ph_of[gi] = ph6

        pm = None

        def emit_mults(gi):
            c, g0, gsz = groups[gi]
            _, sm_ch, _, e_ch = chunk_st[c]
            ph6 = ph_of.pop(gi)
            rhs6 = rhs_p.tile([P, G * RW], F16, tag="rhs6")
            # denominator columns <- e (one strided GpSimd copy per group)
            e_g = e_ch[:, g0 * HEADS:(g0 + gsz) * HEADS]
            den_out = bass.AP(rhs6[:].tensor, rhs6[:].offset + HC,
                              [rhs6[:].ap[0], [RW, gsz], [1, HEADS]])
            nc.scalar.copy(den_out, e_g.rearrange(
                "p (t h) -> p t h", t=gsz))
            # DVE: fused multiply for the first dn tiles
            dn = gsz if gsz <= 2 else gsz - 2
            e_off = g0 * HEADS
            e_base = e_ch[:, e_off:e_off + HEADS]
            e_dve = bass.AP(e_base.tensor, e_base.offset,
                            [e_base.ap[0], [HEADS, dn], [1, HEADS],
                             [0, OUT_DIM]])
            msg_out = bass.AP(rhs6[:].tensor, rhs6[:].offset,
                              [rhs6[:].ap[0], [RW, dn], [OUT_DIM, HEADS],
                               [1, OUT_DIM]])
            nc.vector.tensor_tensor(
                out=msg_out,
                in0=ph6[:, :dn * HC].rearrange(
                    "p (t h c) -> p t h c", t=dn, h=HEADS),
                in1=e_dve, op=mybir.AluOpType.mult)
            # ACT copy + GpSimd multiply for the remaining tiles
            an = gsz - dn
            if an > 0:
                sb2 = rhs_p.tile([P, 2 * HC], F16, tag="sb2")
                nc.scalar.copy(sb2[:, :an * HC],
                               ph6[:, dn * HC:gsz * HC])
                e_base2 = e_ch[:, e_off + dn * HEADS:
                               e_off + dn * HEADS + HEADS]
                e_gps = bass.AP(e_base2.tensor, e_base2.offset,
                                [e_base2.ap[0], [HEADS, an], [1, HEADS],
                                 [0, OUT_DIM]])
                msg_out2 = bass.AP(rhs6[:].tensor,
                                   rhs6[:].offset + dn * RW,
                                   [rhs6[:].ap[0], [RW, an],
                                    [OUT_DIM, HEADS], [1, OUT_DIM]])
                nc.gpsimd.tensor_tensor(
                    out=msg_out2,
                    in0=sb2[:, :an * HC].rearrange(
                        "p (t h c) -> p t h c", t=an, h=HEADS),
                    in1=e_gps, op=mybir.AluOpType.mult)

            rhs_of[gi] = rhs6

        def emit_pm(gi):
            nonlocal pm
            c, g0, gsz = groups[gi]
            _, sm_ch, _, e_ch = chunk_st[c]
            rhs6 = rhs_of.pop(gi)
            # aggregation matmuls (one 260-wide per tile)
            for ti in range(gsz):
                t = c * CH + g0 + ti
                b, first, last = tinfo[t]
                if first:
                    pm = pm_p.tile([P, RW], F32, space="PSUM", tag="pm")
                nc.tensor.matmul(
                    pm[:], lhsT=sm_ch[:, (g0 + ti) * P:(g0 + ti + 1) * P],
                    rhs=rhs6[:, ti * RW:(ti + 1) * RW],
                    start=first, stop=last, skip_group_check=True)
                if last:
                    # head-major planes: acc_h[p, h*NB*64 + b*64 + c]
                    hp_out = bass.AP(
                        acc_h[:].tensor, acc_h[:].offset + b * OUT_DIM,
                        [acc_h[:].ap[0], [NB * OUT_DIM, HEADS],
                         [1, OUT_DIM]])
                    nc.scalar.copy(hp_out, pm[:, 0:HC].rearrange(
                        "p (h c) -> p h c", h=HEADS))
                    nc.scalar.copy(
                        acc_den[:, b * HEADS:(b + 1) * HEADS], pm[:, HC:RW])

        # pipeline: DMA 2 chunks ahead; exp for chunk c+1 emitted just
        # before its first ph; ph one group ahead of the multiplies; the
        # aggregation matmuls lag one group so PE never waits on rhs6.
        emit_dma(0)
        emit_exp(0)
        if nchunks > 1:
            emit_dma(1)
        emit_ph(0)
        for gi in range(len(groups)):
            c = groups[gi][0]
            if gi + 1 < len(groups):
                cn = groups[gi + 1][0]
                if cn != c:
                    if cn + 1 < nchunks:
                        emit_dma(cn + 1)
                    emit_exp(cn)
                emit_ph(gi + 1)
            emit_mults(gi)
            if gi >= 2:
                emit_pm(gi - 2)
        emit_pm(len(groups) - 2)
        emit_pm(len(groups) - 1)

        # ---- batched epilogue, block-split across DVE/GpSimd ----
        B1 = 33                                          # DVE blocks [0,B1)
        W1 = B1 * OUT_DIM
        WT = NB * OUT_DIM
        rec = epi_p.tile([P, NB * HEADS], F32)
        nc.vector.reciprocal(rec[:], acc_den[:])
        nc.vector.tensor_scalar(
            out=rec[:], in0=rec[:], scalar1=1.0 / HEADS, scalar2=None,
            op0=mybir.AluOpType.mult)

        rec16 = epi_p.tile([P, NB * HEADS], F16)
        nc.vector.tensor_copy(rec16[:], rec[:])
        # expand rec per head to [P, NB*OUT_DIM] via DMA replication
        rexp = [epi_p.tile([P, NB * OUT_DIM], F16, tag=f"rx{h}",
                           name=f"rexp{h}") for h in range(HEADS)]
        for h in range(HEADS):
            rb = bass.AP(rec16[:].tensor, rec16[:].offset + h,
                         [rec16[:].ap[0], [HEADS, NB], [0, OUT_DIM]])
            nc.scalar.copy(rexp[h][:], rb)

        # head-mean: all-contiguous elementwise ops, block-split DVE/GpSimd
        macc = epi_p.tile([P, NB * OUT_DIM], F16)
        tmpA = epi_p.tile([P, NB * OUT_DIM], F16, tag="tmpA")
        tmpB = epi_p.tile([P, NB * OUT_DIM], F16, tag="tmpB")
        bias_b = bass.AP(cr_s[:].tensor, cr_s[:].offset,
                         [cr_s[:].ap[0], [0, NB], [1, OUT_DIM]])

        def hm(eng, lo, hi, tmp):
            n = hi - lo
            eng.tensor_tensor(out=macc[:, lo:hi],
                              in0=acc_h[:, lo:hi],
                              in1=rexp[0][:, lo:hi], op=mybir.AluOpType.mult)
            for hd in range(1, HEADS):
                eng.tensor_tensor(
                    out=tmp[:, lo:hi],
                    in0=acc_h[:, hd * WT + lo:hd * WT + hi],
                    in1=rexp[hd][:, lo:hi], op=mybir.AluOpType.mult)
                eng.tensor_add(macc[:, lo:hi], macc[:, lo:hi], tmp[:, lo:hi])
            eng.tensor_tensor(
                out=macc[:, lo:hi], in0=macc[:, lo:hi],
                in1=bass.AP(bias_b.tensor, bias_b.offset,
                            [bias_b.ap[0], [0, n // OUT_DIM], [1, OUT_DIM]]),
                op=mybir.AluOpType.add)

        hm(nc.vector, 0, W1, tmpA)
        hm(nc.gpsimd, W1, WT, tmpB)

        # LayerNorm stats via free-dim reduction (DVE) + Square on ACT
        mean = epi_p.tile([P, NB], F32)
        nc.vector.tensor_reduce(
            out=mean[:], in_=macc[:].rearrange("p (b c) -> p b c", c=OUT_DIM),
            axis=mybir.AxisListType.X, op=mybir.AluOpType.add)
        nc.vector.tensor_scalar(
            out=mean[:], in0=mean[:], scalar1=1.0 / OUT_DIM, scalar2=None,
            op0=mybir.AluOpType.mult)
        sq = epi_p.tile([P, NB * OUT_DIM], F16, tag="tmpA")
        nc.scalar.activation(sq[:], macc[:],
                             mybir.ActivationFunctionType.Square)
        msq = epi_p.tile([P, NB], F32)
        nc.vector.tensor_reduce(
            out=msq[:], in_=sq[:].rearrange("p (b c) -> p b c", c=OUT_DIM),
            axis=mybir.AxisListType.X, op=mybir.AluOpType.add)
        nc.vector.tensor_scalar(
            out=msq[:], in0=msq[:], scalar1=1.0 / OUT_DIM, scalar2=None,
            op0=mybir.AluOpType.mult)
        m2 = epi_p.tile([P, NB], F32)
        nc.vector.tensor_tensor(out=m2[:], in0=mean[:], in1=mean[:],
                                op=mybir.AluOpType.mult)
        var = epi_p.tile([P, NB], F32)
        nc.vector.tensor_tensor(out=var[:], in0=msq[:], in1=m2[:],
                                op=mybir.AluOpType.subtract)

        # rstd = 1 / sqrt(var + eps)
        eps_s = epi_p.tile([P, 1], F32)
        nc.vector.memset(eps_s[:], EPS)
        rstd = epi_p.tile([P, NB], F32)
        nc.scalar.activation(rstd[:], var[:],
                             mybir.ActivationFunctionType.Sqrt,
                             bias=eps_s[:, 0:1])
        nc.vector.reciprocal(rstd[:], rstd[:])

        # expand mean/rstd on ACT, then contiguous normalize split by blocks
        ms16 = epi_p.tile([P, 2 * NB], F16)
        nc.vector.tensor_copy(ms16[:, :NB], mean[:])
        nc.vector.tensor_copy(ms16[:, NB:], rstd[:])
        mexp = epi_p.tile([P, NB * OUT_DIM], F16, tag="rx0")
        mb = bass.AP(ms16[:].tensor, ms16[:].offset,
                     [ms16[:].ap[0], [1, NB], [0, OUT_DIM]])
        nc.scalar.copy(mexp[:], mb)
        sexp = epi_p.tile([P, NB * OUT_DIM], F16, tag="rx1")
        sb = bass.AP(ms16[:].tensor, ms16[:].offset + NB,
                     [ms16[:].ap[0], [1, NB], [0, OUT_DIM]])
        nc.scalar.copy(sexp[:], sb)

        gamma_b = bass.AP(cr_s[:].tensor, cr_s[:].offset + OUT_DIM,
                          [cr_s[:].ap[0], [0, NB], [1, OUT_DIM]])
        beta_b = bass.AP(cr_s[:].tensor, cr_s[:].offset + 2 * OUT_DIM,
                         [cr_s[:].ap[0], [0, NB], [1, OUT_DIM]])

        def norm(eng, lo, hi):
            n = (hi - lo) // OUT_DIM
            eng.tensor_tensor(out=macc[:, lo:hi], in0=macc[:, lo:hi],
                              in1=mexp[:, lo:hi],
                              op=mybir.AluOpType.subtract)
            eng.tensor_tensor(out=macc[:, lo:hi], in0=macc[:, lo:hi],
                              in1=sexp[:, lo:hi], op=mybir.AluOpType.mult)
            eng.tensor_tensor(out=macc[:, lo:hi], in0=macc[:, lo:hi],
                              in1=bass.AP(gamma_b.tensor, gamma_b.offset,
                                          [gamma_b.ap[0], [0, n],
                                           [1, OUT_DIM]]),
                              op=mybir.AluOpType.mult)
            eng.tensor_tensor(out=macc[:, lo:hi], in0=macc[:, lo:hi],
                              in1=bass.AP(beta_b.tensor, beta_b.offset,
                                          [beta_b.ap[0], [0, n],
                                           [1, OUT_DIM]]),
                              op=mybir.AluOpType.add)

        norm(nc.vector, 0, W1)
        norm(nc.gpsimd, W1, WT)

        # PReLU on ACT + store, in two halves so the first DMA overlaps
        pos = epi_p.tile([P, NB * OUT_DIM], F16, tag="rx2")
        nc.scalar.activation(pos[:, :W1], macc[:, :W1],
                             mybir.ActivationFunctionType.Prelu,
                             alpha=w_prelu)
        nc.sync.dma_start(out.ap()[:, :W1], pos[:, :W1])
        nc.scalar.activation(pos[:, W1:], macc[:, W1:],
                             mybir.ActivationFunctionType.Prelu,
                             alpha=w_prelu)
        nc.sync.dma_start(out.ap()[:, W1:], pos[:, W1:])

    nc.compile()
    return nc


def _prep(x, edge_index, W, att_src, att_dst, bias, gamma, beta, prelu_w):
    """Host-side sharding: self-loops, dst-sort, per-core per-block padding,
    per-edge-slot source-feature / logit expansion, fp8 one-hot mask stream,
    weight folding."""
    src = np.concatenate([edge_index[0], np.arange(N, dtype=edge_index.dtype)])
    dst = np.concatenate([edge_index[1], np.arange(N, dtype=edge_index.dtype)])
    order = np.argsort(dst, kind="stable")
    src = src[order].astype(np.int64)
    dst = dst[order].astype(np.int64)

    # folded attention vectors: a_src = x @ V, a_dst = x @ U
    Wh = W.reshape(IN_DIM, HEADS, OUT_DIM)
    V = np.einsum("khc,hc->kh", Wh, att_src)                     # [128, H]
    U = np.einsum("khc,hc->kh", Wh, att_dst)                     # [128, H]

    x16 = x.astype(np.float16)
    a_src_n = x16.astype(np.float32) @ V.astype(np.float16).astype(np.float32)
    a_dst_n = x16.astype(np.float32) @ U.astype(np.float16).astype(np.float32)

    # per-core / per-block edge counts -> shared tile budget T_b
    counts = np.zeros((NCORES, NB), dtype=np.int64)
    core_of = dst // ND
    blk_of = (dst % ND) // P
    np.add.at(counts, (core_of, blk_of), 1)
    T_b = tuple(int(v) for v in np.ceil(counts.max(axis=0) / P).astype(np.int64))
    S = int(sum(T_b)) * P

    in_maps = []
    W16 = W.astype(np.float16)
    crep = np.zeros((P, 3 * OUT_DIM + 1), dtype=np.float32)
    crep[:, 0:OUT_DIM] = bias
    crep[:, OUT_DIM:2 * OUT_DIM] = gamma
    crep[:, 2 * OUT_DIM:3 * OUT_DIM] = beta
    crep[:, 3 * OUT_DIM] = prelu_w[0]

    slot_starts = np.concatenate([[0], np.cumsum(np.array(T_b) * P)])
    eye8 = np.eye(P, dtype=NP_F8)
    for k in range(NCORES):
        sel = core_of == k
        src_k, dst_k = src[sel], dst[sel]
        blk_k = (dst_k % ND) // P

        src_slots = np.zeros(S, dtype=np.int64)
        pad_mask = np.ones(S, dtype=bool)
        dloc = np.full(S, 127, dtype=np.int64)
        dst_slots = np.zeros(S, dtype=np.int64)
        o = np.argsort(blk_k, kind="stable")
        src_k, dst_k, blk_k = src_k[o], dst_k[o], blk_k[o]
        bstart = np.searchsorted(blk_k, np.arange(NB + 1))
        for b in range(NB):
            lo, hi = bstart[b], bstart[b + 1]
            n = hi - lo
            s0 = slot_starts[b]
            src_slots[s0:s0 + n] = src_k[lo:hi]
            pad_mask[s0:s0 + n] = False
            dloc[s0:s0 + n] = (dst_k[lo:hi] % ND) % P
            dst_slots[s0:s0 + n] = dst_k[lo:hi]

        xe = x16[src_slots]                          # [S, 128]
        xe[pad_mask] = np.float16(0)
        xeT = np.ascontiguousarray(xe.T)             # [128, S]

        alpha = (a_src_n[src_slots] + a_dst_n[dst_slots]).astype(np.float16)
        alpha[pad_mask] = np.float16(-30000.0)
        # layout [e, (t, h)]: partition = edge-in-tile
        alphaT = np.ascontiguousarray(
            alpha.reshape(S // P, P, HEADS).transpose(1, 0, 2).reshape(
                P, (S // P) * HEADS))

        # one-hot mask, tile-major along free dim, fp8
        oh = eye8[dloc].reshape(S // P, P, P)        # [t, e, d]
        smask = np.ascontiguousarray(
            oh.transpose(1, 0, 2).reshape(P, S))     # [e, (t d)]

        in_maps.append({
            "xeT": xeT, "smask": smask, "alphaT": alphaT,
            "W16": W16, "crep": crep,
        })
    return S, T_b, in_maps


def kernel(x, edge_index, W, att_src, att_dst, bias, gamma, beta, prelu_w,
           _trace=False):
    x = np.asarray(x, dtype=np.float32)
    edge_index = np.asarray(edge_index)
    S, T_b, in_maps = _prep(
        x, edge_index, np.asarray(W, np.float32), np.asarray(att_src, np.float32),
        np.asarray(att_dst, np.float32), np.asarray(bias, np.float32),
        np.asarray(gamma, np.float32), np.asarray(beta, np.float32),
        np.asarray(prelu_w, np.float32))

    key = (S, T_b)
    if key not in _CACHE:
        _CACHE[key] = _build(S, T_b)
    nc = _CACHE[key]

    res = run_bass_kernel_spmd(nc, in_maps, core_ids=list(range(NCORES)),
                               trace=_trace)
    outs = []
    for k in range(NCORES):
        dump = res.results[k]["out"].astype(np.float32).reshape(P, NB, OUT_DIM)
        outs.append(dump.transpose(1, 0, 2).reshape(NDP, OUT_DIM)[:ND])
    out = np.concatenate(outs, axis=0)
    if _trace:
        kernel.last_exec_time_ns = res.exec_time_ns
    return out


# revision 26
# speedup vs baseline: 1.1818x; 1.1818x over previous
"""GAT layer (project + edge-softmax attention + aggregate + head-mean + LayerNorm + PReLU)
on 8 Trainium2 NeuronCores.

Sharding: nodes/edges partitioned by destination across the 8 cores; edges of
each core are grouped into 128-destination blocks and 128-edge tiles.

Device pipeline (v4):
 - host ships, per edge slot: source features x (fp16, the 99.9%-of-FLOPs
   projection runs on device), the fp8 one-hot destination mask (exact 0/1),
   and the folded attention logit a_src+a_dst (fp16, 8B) pre-gathered the same
   way the features are.
 - per 48-tile chunk: one DVE leaky-relu + one ACT exp produce the edge exp
   weights.
 - per 6-tile PSUM group: 6 projection matmuls (xet.T @ W, fp16); the h*e
   multiply splits DVE (4 tiles, fused PSUM->SBUF) / ACT copy + GpSimd
   multiply (2 tiles); softmax denominators ride as 4 extra rhs columns
   (strided GpSimd copy of e).
 - per tile one 260-wide aggregation matmul (fp8 mask stationary x fp16
   moving) accumulating per dst block in a double-buffered PSUM bank; ACT
   copies finished blocks out.
 - the tail epilogue (head-mean with per-(dst,head) softmax denominators,
   LayerNorm, PReLU) splits across DVE/GpSimd by block range, with ACT doing
   the broadcasts/exp-like ops; output is written contiguously and
   de-interleaved on the host.
"""
import sys

sys.path.insert(0, "/opt/trn_rl_repo")

import numpy as np
from contextlib import ExitStack

import concourse.bass as bass
import concourse.tile as tile
from concourse import bacc, mybir
from concourse.bass_utils import run_bass_kernel_spmd

# ---- problem constants (hardcoded per harness contract) ----
N = 50000
IN_DIM = 128
OUT_DIM = 64
HEADS = 4
HC = HEADS * OUT_DIM          # 256
NEG_SLOPE = 0.2
EPS = 1e-5

NCORES = 8
ND = N // NCORES              # 6250 dst nodes per core
P = 128
NB = (ND + P - 1) // P        # 49 blocks (last has 106 dsts)
NDP = NB * P                  # 6272 padded local nodes
G = 6                         # tiles per PSUM projection group (3 banks)
CH = 48                       # tiles per alpha chunk (multiple of G)

F8 = mybir.dt.float8e4
F16 = mybir.dt.float16
F32 = mybir.dt.float32
NP_F8 = mybir.dt.np(F8)

_CACHE = {}


def _build(S, T_b):
    """Compile the SPMD program. S = padded edge slots per core (mult of 128),
    T_b = tuple of per-block tile counts (len NB, sum*128 == S)."""
    n_tiles = S // P
    RW = HC + HEADS           # 260 psum width (256 msg + 4 denom cols)

    nc = bacc.Bacc("TRN2", target_bir_lowering=False, debug=False)

    xeT = nc.dram_tensor("xeT", [P, S], F16, kind="ExternalInput")
    smaskd = nc.dram_tensor("smask", [P, S], F8, kind="ExternalInput")
    alphad = nc.dram_tensor("alphaT", [P, (S // P) * HEADS], F16,
                            kind="ExternalInput")
    W16d = nc.dram_tensor("W16", [P, HC], F16, kind="ExternalInput")
    # packed per-channel constants replicated across partitions:
    # [bias(64) | gamma(64) | beta(64) | prelu_w(1)]
    crep = nc.dram_tensor("crep", [P, 3 * OUT_DIM + 1], F32, kind="ExternalInput")
    # contiguous output dump [p, b, c]; host de-interleaves
    out = nc.dram_tensor("out", [P, NB * OUT_DIM], F16, kind="ExternalOutput")

    # tile -> (block, is_first_in_block, is_last_in_block)
    tinfo = []
    for b, nt in enumerate(T_b):
        for ti in range(nt):
            tinfo.append((b, ti == 0, ti == nt - 1))

    with tile.TileContext(nc) as tc, ExitStack() as ctx:
        const_p = ctx.enter_context(tc.tile_pool(name="const", bufs=1))
        xet_p = ctx.enter_context(tc.tile_pool(name="xet", bufs=2))
        rhs_p = ctx.enter_context(tc.tile_pool(name="rhs", bufs=4))
        ach_p = ctx.enter_context(tc.tile_pool(name="ach", bufs=2))
        epi_p = ctx.enter_context(tc.tile_pool(name="epi", bufs=1))
        ph_p = ctx.enter_context(tc.tile_pool(name="ph", bufs=2, space="PSUM"))
        pm_p = ctx.enter_context(tc.tile_pool(name="pm", bufs=2, space="PSUM"))

        # ---- constants ----
        w_s = const_p.tile([P, HC], F16)
        nc.sync.dma_start(w_s[:], W16d[:])
        cr_s = const_p.tile([P, 3 * OUT_DIM + 1], F32)
        nc.sync.dma_start(cr_s[:], crep[:])
        w_prelu = cr_s[:, 3 * OUT_DIM:3 * OUT_DIM + 1]

        # big accumulators for the batched epilogue (head-major for
        # contiguous tail ops)
        acc_h = const_p.tile([P, HEADS * NB * OUT_DIM], F16)
        acc_den = const_p.tile([P, NB * HEADS], F16)

        # ---- main loop (software-pipelined) ----
        nchunks = (n_tiles + CH - 1) // CH

        # groups spanning all chunks: (chunk, tile offset in chunk, size)
        groups = []
        for c in range(nchunks):
            ctiles = min(CH, n_tiles - c * CH)
            for g0 in range(0, ctiles, G):
                groups.append((c, g0, min(G, ctiles - g0)))

        chunk_st = {}

        def emit_dma(c):
            ctiles = min(CH, n_tiles - c * CH)
            lo, hi = c * CH * P, (c * CH + ctiles) * P
            w = hi - lo
            xet_ch = xet_p.tile([P, CH * P], F16, tag="xet")
            nc.sync.dma_start(xet_ch[:, :w], xeT[:, lo:hi])
            sm_ch = xet_p.tile([P, CH * P], F8, tag="smask")
            nc.sync.dma_start(sm_ch[:, :w], smaskd[:, lo:hi])
            al_ch = xet_p.tile([P, CH * HEADS], F16, tag="alpha")
            nc.sync.dma_start(al_ch[:, :ctiles * HEADS],
                              alphad[:, c * CH * HEADS:
                                     (c * CH + ctiles) * HEADS])
            chunk_st[c] = [xet_ch, sm_ch, al_ch, None]

        def emit_exp(c):
            ctiles = min(CH, n_tiles - c * CH)
            al_ch = chunk_st[c][2]
            cw = ctiles * HEADS
            lk_ch = ach_p.tile([P, CH * HEADS], F32, tag="lk_ch")
            nc.vector.scalar_tensor_tensor(
                out=lk_ch[:, :cw], in0=al_ch[:, :cw],
                scalar=NEG_SLOPE, in1=al_ch[:, :cw],
                op0=mybir.AluOpType.mult, op1=mybir.AluOpType.max)
            e_ch = ach_p.tile([P, CH * HEADS], F16, tag="e_ch")
            nc.scalar.activation(e_ch[:, :cw], lk_ch[:, :cw],
                                 mybir.ActivationFunctionType.Exp)
            chunk_st[c][3] = e_ch

        ph_of = {}
        rhs_of = {}

        def emit_ph(gi):
            c, g0, gsz = groups[gi]
            xet_ch = chunk_st[c][0]
            ph6 = ph_p.tile([P, G * HC], F32, space="PSUM", tag="ph6")
            for ti in range(gsz):
                nc.tensor.matmul(
                    ph6[:, ti * HC:(ti + 1) * HC],
                    lhsT=xet_ch[:, (g0 + ti) * P:(g0 + ti + 1) * P],
                    rhs=w_s[:], start=True, stop=True,
                    skip_group_check=True)
            ph_of[gi] = ph6

        pm = None

        def emit_mults(gi):
            c, g0, gsz = groups[gi]
            _, sm_ch, _, e_ch = chunk_st[c]
            ph6 = ph_of.pop(gi)
            rhs6 = rhs_p.tile([P, G * RW], F16, tag="rhs6")
            # denominator columns <- e (one strided GpSimd copy per group)
            e_g = e_ch[:, g0 * HEADS:(g0 + gsz) * HEADS]
            den_out = bass.AP(rhs6[:].tensor, rhs6[:].offset + HC,
                              [rhs6[:].ap[0], [RW, gsz], [1, HEADS]])
            nc.scalar.copy(den_out, e_g.rearrange(
                "p (t h) -> p t h", t=gsz))
            # DVE: fused multiply for the first dn tiles
            dn = gsz if gsz <= 2 else gsz - 2
            e_off = g0 * HEADS
            e_base = e_ch[:, e_off:e_off + HEADS]
            e_dve = bass.AP(e_base.tensor, e_base.offset,
                            [e_base.ap[0], [HEADS, dn], [1, HEADS],
                             [0, OUT_DIM]])
            msg_out = bass.AP(rhs6[:].tensor, rhs6[:].offset,
                              [rhs6[:].ap[0], [RW, dn], [OUT_DIM, HEADS],
                               [1, OUT_DIM]])
            nc.vector.tensor_tensor(
                out=msg_out,
                in0=ph6[:, :dn * HC].rearrange(
                    "p (t h c) -> p t h c", t=dn, h=HEADS),
                in1=e_dve, op=mybir.AluOpType.mult)
            # ACT copy + GpSimd multiply for the remaining tiles
            an = gsz - dn
            if an > 0:
                sb2 = rhs_p.tile([P, 2 * HC], F16, tag="sb2")
                nc.scalar.copy(sb2[:, :an * HC],
                               ph6[:, dn * HC:gsz * HC])
                e_base2 = e_ch[:, e_off + dn * HEADS:
                               e_off + dn * HEADS + HEADS]
                e_gps = bass.AP(e_base2.tensor, e_base2.offset,
                                [e_base2.ap[0], [HEADS, an], [1, HEADS],
                                 [0, OUT_DIM]])
                msg_out2 = bass.AP(rhs6[:].tensor,
                                   rhs6[:].offset + dn * RW,
                                   [rhs6[:].ap[0], [RW, an],
                                    [OUT_DIM, HEADS], [1, OUT_DIM]])
                nc.gpsimd.tensor_tensor(
                    out=msg_out2,
                    in0=sb2[:, :an * HC].rearrange(
                        "p (t h c) -> p t h c", t=an, h=HEADS),
                    in1=e_gps, op=mybir.AluOpType.mult)

            rhs_of[gi] = rhs6

        def emit_pm(gi):
            nonlocal pm
            c, g0, gsz = groups[gi]
            _, sm_ch, _, e_ch = chunk_st[c]
            rhs6 = rhs_of.pop(gi)
            # aggregation matmuls (one 260-wide per tile)
            for ti in range(gsz):
                t = c * CH + g0 + ti
                b, first, last = tinfo[t]
                if first:
                    pm = pm_p.tile([P, RW], F32, space="PSUM", tag="pm")
                nc.tensor.matmul(
                    pm[:], lhsT=sm_ch[:, (g0 + ti) * P:(g0 + ti + 1) * P],
                    rhs=rhs6[:, ti * RW:(ti + 1) * RW],
                    start=first, stop=last, skip_group_check=True)
                if last:
                    # head-major planes: acc_h[p, h*NB*64 + b*64 + c]
                    hp_out = bass.AP(
                        acc_h[:].tensor, acc_h[:].offset + b * OUT_DIM,
                        [acc_h[:].ap[0], [NB * OUT_DIM, HEADS],
                         [1, OUT_DIM]])
                    nc.scalar.copy(hp_out, pm[:, 0:HC].rearrange(
                        "p (h c) -> p h c", h=HEADS))
                    nc.scalar.copy(
                        acc_den[:, b * HEADS:(b + 1) * HEADS], pm[:, HC:RW])

        # pipeline: DMA 2 chunks ahead; exp for chunk c+1 emitted just
        # before its first ph; ph one group ahead of the multiplies; the
        # aggregation matmuls lag one group so PE never waits on rhs6.
        emit_dma(0)
        emit_exp(0)
        if nchunks > 1:
            emit_dma(1)
        emit_ph(0)
        for gi in range(len(groups)):
            c = groups[gi][0]
            if gi + 1 < len(groups):
                cn = groups[gi + 1][0]
                if cn != c:
                    if cn + 1 < nchunks:
                        emit_dma(cn + 1)
                    emit_exp(cn)
                emit_ph(gi + 1)
            emit_mults(gi)
            if gi >= 2:
                emit_pm(gi - 2)
        emit_pm(len(groups) - 2)
        emit_pm(len(groups) - 1)

        # ---- batched epilogue, block-split across DVE/GpSimd ----
        B1 = 29                                          # DVE blocks [0,B1)
        W1 = B1 * OUT_DIM
        WT = NB * OUT_DIM
        rec = epi_p.tile([P, NB * HEADS], F32)
        nc.vector.reciprocal(rec[:], acc_den[:])
        nc.vector.tensor_scalar(
            out=rec[:], in0=rec[:], scalar1=1.0 / HEADS, scalar2=None,
            op0=mybir.AluOpType.mult)

        rec16 = epi_p.tile([P, NB * HEADS], F16)
        nc.vector.tensor_copy(rec16[:], rec[:])
        # expand rec per head to [P, NB*OUT_DIM] via DMA replication
        rexp = [epi_p.tile([P, NB * OUT_DIM], F16, tag=f"rx{h}",
                           name=f"rexp{h}") for h in range(HEADS)]
        for h in range(HEADS):
            rb = bass.AP(rec16[:].tensor, rec16[:].offset + h,
                         [rec16[:].ap[0], [HEADS, NB], [0, OUT_DIM]])
            nc.scalar.copy(rexp[h][:], rb)

        # head-mean: all-contiguous elementwise ops, block-split DVE/GpSimd
        macc = epi_p.tile([P, NB * OUT_DIM], F16)
        tmpA = epi_p.tile([P, NB * OUT_DIM], F16, tag="tmpA")
        tmpB = epi_p.tile([P, NB * OUT_DIM], F16, tag="tmpB")
        bias_b = bass.AP(cr_s[:].tensor, cr_s[:].offset,
                         [cr_s[:].ap[0], [0, NB], [1, OUT_DIM]])

        def hm(eng, lo, hi, tmp):
            n = hi - lo
            eng.tensor_tensor(out=macc[:, lo:hi],
                              in0=acc_h[:, lo:hi],
                              in1=rexp[0][:, lo:hi], op=mybir.AluOpType.mult)
            for hd in range(1, HEADS):
                eng.tensor_tensor(
                    out=tmp[:, lo:hi],
                    in0=acc_h[:, hd * WT + lo:hd * WT + hi],
                    in1=rexp[hd][:, lo:hi], op=mybir.AluOpType.mult)
                eng.tensor_add(macc[:, lo:hi], macc[:, lo:hi], tmp[:, lo:hi])
            eng.tensor_tensor(
                out=macc[:, lo:hi], in0=macc[:, lo:hi],
                in1=bass.AP(bias_b.tensor, bias_b.offset,
                            [bias_b.ap[0], [0, n // OUT_DIM], [1, OUT_DIM]]),
                op=mybir.AluOpType.add)

        hm(nc.vector, 0, W1, tmpA)
        hm(nc.gpsimd, W1, WT, tmpB)

        # LayerNorm stats via free-dim reduction (DVE) + Square on ACT
        mean = epi_p.tile([P, NB], F32)
        nc.vector.tensor_reduce(
            out=mean[:], in_=macc[:].rearrange("p (b c) -> p b c", c=OUT_DIM),
            axis=mybir.AxisListType.X, op=mybir.AluOpType.add)
        nc.vector.tensor_scalar(
            out=mean[:], in0=mean[:], scalar1=1.0 / OUT_DIM, scalar2=None,
            op0=mybir.AluOpType.mult)
        sq = epi_p.tile([P, NB * OUT_DIM], F16, tag="tmpA")
        nc.scalar.activation(sq[:], macc[:],
                             mybir.ActivationFunctionType.Square)
        msq = epi_p.tile([P, NB], F32)
        nc.vector.tensor_reduce(
            out=msq[:], in_=sq[:].rearrange("p (b c) -> p b c", c=OUT_DIM),
            axis=mybir.AxisListType.X, op=mybir.AluOpType.add)
        nc.vector.tensor_scalar(
            out=msq[:], in0=msq[:], scalar1=1.0 / OUT_DIM, scalar2=None,
            op0=mybir.AluOpType.mult)
        m2 = epi_p.tile([P, NB], F32)
        nc.vector.tensor_tensor(out=m2[:], in0=mean[:], in1=mean[:],
                                op=mybir.AluOpType.mult)
        var = epi_p.tile([P, NB], F32)
        nc.vector.tensor_tensor(out=var[:], in0=msq[:], in1=m2[:],
                                op=mybir.AluOpType.subtract)

        # rstd = 1 / sqrt(var + eps)
        eps_s = epi_p.tile([P, 1], F32)
        nc.vector.memset(eps_s[:], EPS)
        rstd = epi_p.tile([P, NB], F32)
        nc.scalar.activation(rstd[:], var[:],
                             mybir.ActivationFunctionType.Sqrt,
                             bias=eps_s[:, 0:1])
        nc.vector.reciprocal(rstd[:], rstd[:])

        # expand mean/rstd on ACT, then contiguous normalize split by blocks
        ms16 = epi_p.tile([P, 2 * NB], F16)
        nc.vector.tensor_copy(ms16[:, :NB], mean[:])
        nc.vector.tensor_copy(ms16[:, NB:], rstd[:])
        mexp = epi_p.tile([P, NB * OUT_DIM], F16, tag="rx0")
        mb = bass.AP(ms16[:].tensor, ms16[:].offset,
                     [ms16[:].ap[0], [1, NB], [0, OUT_DIM]])
        nc.scalar.copy(mexp[:], mb)
        sexp = epi_p.tile([P, NB * OUT_DIM], F16, tag="rx1")
        sb = bass.AP(ms16[:].tensor, ms16[:].offset + NB,
                     [ms16[:].ap[0], [1, NB], [0, OUT_DIM]])
        nc.scalar.copy(sexp[:], sb)

        gamma_b = bass.AP(cr_s[:].tensor, cr_s[:].offset + OUT_DIM,
                          [cr_s[:].ap[0], [0, NB], [1, OUT_DIM]])
        beta_b = bass.AP(cr_s[:].tensor, cr_s[:].offset + 2 * OUT_DIM,
                         [cr_s[:].ap[0], [0, NB], [1, OUT_DIM]])

        def norm(eng, lo, hi):
            n = (hi - lo) // OUT_DIM
            eng.tensor_tensor(out=macc[:, lo:hi], in0=macc[:, lo:hi],
                              in1=mexp[:, lo:hi],
                              op=mybir.AluOpType.subtract)
            eng.tensor_tensor(out=macc[:, lo:hi], in0=macc[:, lo:hi],
                              in1=sexp[:, lo:hi], op=mybir.AluOpType.mult)
            eng.tensor_tensor(out=macc[:, lo:hi], in0=macc[:, lo:hi],
                              in1=bass.AP(gamma_b.tensor, gamma_b.offset,
                                          [gamma_b.ap[0], [0, n],
                                           [1, OUT_DIM]]),
                              op=mybir.AluOpType.mult)
            eng.tensor_tensor(out=macc[:, lo:hi], in0=macc[:, lo:hi],
                              in1=bass.AP(beta_b.tensor, beta_b.offset,
                                          [beta_b.ap[0], [0, n],
                                           [1, OUT_DIM]]),
                              op=mybir.AluOpType.add)

        norm(nc.vector, 0, W1)
        norm(nc.gpsimd, W1, WT)

        # PReLU on ACT + store, in two halves so the first DMA overlaps
        pos = epi_p.tile([P, NB * OUT_DIM], F16, tag="rx2")
        nc.scalar.activation(pos[:, :W1], macc[:, :W1],
                             mybir.ActivationFunctionType.Prelu,
                             alpha=w_prelu)
        nc.sync.dma_start(out.ap()[:, :W1], pos[:, :W1])
        nc.scalar.activation(pos[:, W1:], macc[:, W1:],
                             mybir.ActivationFunctionType.Prelu,
                             alpha=w_prelu)
        nc.sync.dma_start(out.ap()[:, W1:], pos[:, W1:])

    nc.compile()
    return nc


def _prep(x, edge_index, W, att_src, att_dst, bias, gamma, beta, prelu_w):
    """Host-side sharding: self-loops, dst-sort, per-core per-block padding,
    per-edge-slot source-feature / logit expansion, fp8 one-hot mask stream,
    weight folding."""
    src = np.concatenate([edge_index[0], np.arange(N, dtype=edge_index.dtype)])
    dst = np.concatenate([edge_index[1], np.arange(N, dtype=edge_index.dtype)])
    order = np.argsort(dst, kind="stable")
    src = src[order].astype(np.int64)
    dst = dst[order].astype(np.int64)

    # folded attention vectors: a_src = x @ V, a_dst = x @ U
    Wh = W.reshape(IN_DIM, HEADS, OUT_DIM)
    V = np.einsum("khc,hc->kh", Wh, att_src)                     # [128, H]
    U = np.einsum("khc,hc->kh", Wh, att_dst)                     # [128, H]

    x16 = x.astype(np.float16)
    a_src_n = x16.astype(np.float32) @ V.astype(np.float16).astype(np.float32)
    a_dst_n = x16.astype(np.float32) @ U.astype(np.float16).astype(np.float32)

    # per-core / per-block edge counts -> shared tile budget T_b
    counts = np.zeros((NCORES, NB), dtype=np.int64)
    core_of = dst // ND
    blk_of = (dst % ND) // P
    np.add.at(counts, (core_of, blk_of), 1)
    T_b = tuple(int(v) for v in np.ceil(counts.max(axis=0) / P).astype(np.int64))
    S = int(sum(T_b)) * P

    in_maps = []
    W16 = W.astype(np.float16)
    crep = np.zeros((P, 3 * OUT_DIM + 1), dtype=np.float32)
    crep[:, 0:OUT_DIM] = bias
    crep[:, OUT_DIM:2 * OUT_DIM] = gamma
    crep[:, 2 * OUT_DIM:3 * OUT_DIM] = beta
    crep[:, 3 * OUT_DIM] = prelu_w[0]

    slot_starts = np.concatenate([[0], np.cumsum(np.array(T_b) * P)])
    eye8 = np.eye(P, dtype=NP_F8)
    for k in range(NCORES):
        sel = core_of == k
        src_k, dst_k = src[sel], dst[sel]
        blk_k = (dst_k % ND) // P

        src_slots = np.zeros(S, dtype=np.int64)
        pad_mask = np.ones(S, dtype=bool)
        dloc = np.full(S, 127, dtype=np.int64)
        dst_slots = np.zeros(S, dtype=np.int64)
        o = np.argsort(blk_k, kind="stable")
        src_k, dst_k, blk_k = src_k[o], dst_k[o], blk_k[o]
        bstart = np.searchsorted(blk_k, np.arange(NB + 1))
        for b in range(NB):
            lo, hi = bstart[b], bstart[b + 1]
            n = hi - lo
            s0 = slot_starts[b]
            src_slots[s0:s0 + n] = src_k[lo:hi]
            pad_mask[s0:s0 + n] = False
            dloc[s0:s0 + n] = (dst_k[lo:hi] % ND) % P
            dst_slots[s0:s0 + n] = dst_k[lo:hi]

        xe = x16[src_slots]                          # [S, 128]
        xe[pad_mask] = np.float16(0)
        xeT = np.ascontiguousarray(xe.T)             # [128, S]

        alpha = (a_src_n[src_slots] + a_dst_n[dst_slots]).astype(np.float16)
        alpha[pad_mask] = np.float16(-30000.0)
        # layout [e, (t, h)]: partition = edge-in-tile
        alphaT = np.ascontiguousarray(
            alpha.reshape(S // P, P, HEADS).transpose(1, 0, 2).reshape(
                P, (S // P) * HEADS))

        # one-hot mask, tile-major along free dim, fp8
        oh = eye8[dloc].reshape(S // P, P, P)        # [t, e, d]
        smask = np.ascontiguousarray(
            oh.transpose(1, 0, 2).reshape(P, S))     # [e, (t d)]

        in_maps.append({
            "xeT": xeT, "smask": smask, "alphaT": alphaT,
            "W16": W16, "crep": crep,
        })
    return S, T_b, in_maps


def kernel(x, edge_index, W, att_src, att_dst, bias, gamma, beta, prelu_w,
           _trace=False):
    x = np.asarray(x, dtype=np.float32)
    edge_index = np.asarray(edge_index)
    S, T_b, in_maps = _prep(
        x, edge_index, np.asarray(W, np.float32), np.asarray(att_src, np.float32),
        np.asarray(att_dst, np.float32), np.asarray(bias, np.float32),
        np.asarray(gamma, np.float32), np.asarray(beta, np.float32),
        np.asarray(prelu_w, np.float32))

    key = (S, T_b)
    if key not in _CACHE:
        _CACHE[key] = _build(S, T_b)
    nc = _CACHE[key]

    res = run_bass_kernel_spmd(nc, in_maps, core_ids=list(range(NCORES)),
                               trace=_trace)
    outs = []
    for k in range(NCORES):
        dump = res.results[k]["out"].astype(np.float32).reshape(P, NB, OUT_DIM)
        outs.append(dump.transpose(1, 0, 2).reshape(NDP, OUT_DIM)[:ND])
    out = np.concatenate(outs, axis=0)
    if _trace:
        kernel.last_exec_time_ns = res.exec_time_ns
    return out
